# revision 9
# baseline (speedup 1.0000x reference)
"""Trainium2 Bass kernel for nn_CTNNJastrow (GNN message passing Jastrow factor).

Strategy
--------
Pure data parallel: batch dim B=4096 split across 8 NeuronCores (512 walkers
per core).  The tiny MLP weights are replicated (and heavily *folded* on the
host first - see below); no cross-core communication.

Math folding (exact linear algebra, done in float64 on host):
The reference network is, per step, a linear edge-update MLP wrapped in one
tanh, plus a linear node-update MLP wrapped in one tanh.  Every linear op
that is NOT separated from another linear op by a tanh can be fused:

  step0 edge pre-activation  c0  = u0*r1 + (P0x @ x)_i + (Q0x @ x)_j  + b0
  h1' = tanh(c0)                                  (only edge-sized state)
  node pre-act  = M0n @ (sum_j h1') + N1a0x @ x + const0 ;  z = tanh(.)
  step1 edge pre-activation  c1  = A1t @ h1' + (P1 @ z)_i + (Q1 @ z)_j + b1
  h2' = tanh(c1)
  node pre-act  = M1n @ (sum_j h2') + N1az @ z + const2 ;  z2 = tanh(.)
  out = w_f2 @ tanh(F1 @ (sum_i z2) + bf1f) + b_f2 + cusp

so per message-passing step only ONE edge-sized (B*N*N, 32) tanh and three
accumulating matmul streams are needed; eu2 / e2v / v2e / node MLPs all
collapse into 32x32 folded matrices applied at node granularity.

Device layout: walkers are processed in tiles of 128.  Edge tensors use a
"4-group stacked" layout [128 partitions = (group g=w//32, feature f=0..31),
free = (w%32, i, j) = 8192 cols] so all engines run with full 128-partition
utilization.  Folded weight matrices are block-diagonalized (4 copies of the
32x32 block) on the host.  The i/j broadcasts are free-dim step-0 access
patterns read directly by the PE as the moving operand; the r1 rank-1 term
enters through a K=4 matmul.

The electron-electron cusp is computed per-walker (partition = walker,
free = 256 edge pairs) with a fused multiply-reduce; the spin-dependent
gamma matrix is precomputed on host from the runtime `spin` input.
"""

import numpy as np

import concourse.bass as bass
import concourse.tile as tile
from concourse import bacc, mybir
from concourse.bass_utils import run_bass_kernel_spmd

F32 = mybir.dt.float32
AF = mybir.ActivationFunctionType
ALU = mybir.AluOpType
AX = mybir.AxisListType

# Problem constants (fixed by the module definition)
N_PART = 16
D = 2
NH = 32
EH = 32
RH = 16
B_FULL = 4096
N_CORES = 8
B_CORE = B_FULL // N_CORES      # 512 walkers per core
WT = 128                        # walkers per on-chip tile
N_TILES = B_CORE // WT          # 4
G = 4                           # partition stacking groups (of 32 walkers)
W32 = WT // G                   # 32
ECOLS = W32 * N_PART * N_PART   # 8192 stacked edge cols per tile
VCOLS = W32 * N_PART            # 512 stacked node cols per tile
CHUNK = 512                     # psum bank / fp32 moving-operand limit
N_CHUNKS = ECOLS // CHUNK       # 16


# ----------------------------------------------------------------------------
# Host-side weight folding (float64 for accuracy, cast to f32 at the end)
# ----------------------------------------------------------------------------

def _blkdiag(block: np.ndarray) -> np.ndarray:
    """Replicate a [kb, mb] lhsT block on the diagonal 4x -> [4*kb, 4*mb]."""
    kb, mb = block.shape
    out = np.zeros((G * kb, G * mb), np.float64)
    for g in range(G):
        out[g * kb:(g + 1) * kb, g * mb:(g + 1) * mb] = block
    return out


def _blkdiag_dg(block: np.ndarray) -> np.ndarray:
    """lhsT for K-rows laid out as (d, g) [d-major] instead of (g, d).

    block: [D, mb].  Returns [D*4, 4*mb] with out[d*4+g, g*mb:(g+1)*mb] =
    block[d].  Matches an xT tile whose partitions are d*4+g.
    """
    db, mb = block.shape
    out = np.zeros((db * G, G * mb), np.float64)
    for d in range(db):
        for g in range(G):
            out[d * G + g, g * mb:(g + 1) * mb] = block[d]
    return out


def _fold_params(inp: dict) -> dict:
    f = lambda k: np.asarray(inp[k], np.float64)
    w_ne, w_ee = f("w_ne"), f("w_ee")
    w_v2e, w_eu1, b_eu1 = f("w_v2e"), f("w_eu1"), f("b_eu1")
    w_eu2, b_eu2, w_e2v = f("w_eu2"), f("b_eu2"), f("w_e2v")
    w_nu1, b_nu1, w_nu2, b_nu2 = f("w_nu1"), f("b_nu1"), f("w_nu2"), f("b_nu2")
    w_f1, b_f1, w_f2, b_f2 = f("w_f1"), f("b_f1"), f("w_f2"), f("b_f2")
    spin = np.asarray(inp["spin"])

    A0, B0, C0 = w_eu1[0][:, :EH], w_eu1[0][:, EH:2 * EH], w_eu1[0][:, 2 * EH:]
    A1, B1, C1 = w_eu1[1][:, :EH], w_eu1[1][:, EH:2 * EH], w_eu1[1][:, 2 * EH:]
    w_r, w_i, w_j = w_ee[:, 0:1], w_ee[:, 1:1 + NH], w_ee[:, 1 + NH:]

    # step 0 edge pre-activation: c0 = u0*r1 + (P0x x)_i + (Q0x x)_j + b_eu1[0]
    u0 = A0 @ w_r                                   # [EH, 1]
    P0x = (A0 @ w_i + B0 @ w_v2e[0]) @ w_ne         # [EH, D]
    Q0x = (A0 @ w_j + C0 @ w_v2e[0]) @ w_ne         # [EH, D]

    # step 0 node update (on s0 = sum_j h1' and x):
    N1a0, N1b0 = w_nu1[0][:, :NH], w_nu1[0][:, NH:]
    M0n = N1b0 @ (w_e2v[0] @ w_eu2[0])              # [NH, EH]
    N1a0x = N1a0 @ w_ne                             # [NH, D]
    const0 = N1b0 @ (16.0 * w_e2v[0] @ b_eu2[0]) + b_nu1[0]

    # step 1 edge pre-activation: c1 = A1t h1' + (P1 z)_i + (Q1 z)_j + bias1
    A1t = A1 @ w_eu2[0]                             # [EH, EH]
    P1 = B1 @ w_v2e[1] @ w_nu2[0]                   # [EH, NH]
    Q1 = C1 @ w_v2e[1] @ w_nu2[0]                   # [EH, NH]
    bias1 = b_eu1[1] + A1 @ b_eu2[0] + (B1 + C1) @ (w_v2e[1] @ b_nu2[0])

    # step 1 node update (on s1 = sum_j h2' and z):
    N1a1, N1b1 = w_nu1[1][:, :NH], w_nu1[1][:, NH:]
    M1n = N1b1 @ (w_e2v[1] @ w_eu2[1])              # [NH, EH]
    N1az = N1a1 @ w_nu2[0]                          # [NH, NH]
    const2 = N1a1 @ b_nu2[0] + N1b1 @ (16.0 * w_e2v[1] @ b_eu2[1]) + b_nu1[1]

    # readout (on sum_i z2):
    F1 = (w_f1 @ w_nu2[1]) / 16.0                   # [RH, NH]
    bf1f = w_f1 @ b_nu2[1] + b_f1                   # [RH]

    # cusp: gamma over ALL ordered pairs, 0.1 factor (0.2 * triu->full/2),
    # diagonal zeroed. gamma_para = 1/(D+1), gamma_apara = 1/(D-1).
    same = (spin[:, None] == spin[None, :]).astype(np.float64)
    gamma = same * (1.0 / (D + 1)) + (1.0 - same) * (1.0 / (D - 1))
    Gd = 0.1 * gamma
    np.fill_diagonal(Gd, 0.0)
    csum = Gd.sum() + float(b_f2[0])

    rep4 = lambda v: np.tile(np.asarray(v, np.float64), G)[:, None]
    t32 = lambda a: np.ascontiguousarray(a, np.float32)

    return {
        "u0blk": t32(_blkdiag(u0.T)),         # [4, 128]
        "p0xblk": t32(_blkdiag_dg(P0x.T)),    # [8, 128] (d,g) rows
        "q0xblk": t32(_blkdiag_dg(Q0x.T)),    # [8, 128] (d,g) rows
        "a1blk": t32(_blkdiag(A1t.T)),        # [128, 128]
        "p1blk": t32(_blkdiag(P1.T)),         # [128, 128]
        "q1blk": t32(_blkdiag(Q1.T)),         # [128, 128]
        "m0nblk": t32(_blkdiag(M0n.T)),       # [128, 128]
        "n1a0xblk": t32(_blkdiag_dg(N1a0x.T)),  # [8, 128] (d,g) rows
        "m1nblk": t32(_blkdiag(M1n.T)),       # [128, 128]
        "n1azblk": t32(_blkdiag(N1az.T)),     # [128, 128]
        "f1blk": t32(_blkdiag(F1.T)),         # [128, 64]
        "wf2blk": t32(_blkdiag(w_f2.T)),      # [64, 4]
        "b0rep": t32(rep4(b_eu1[0])),         # [128, 1]
        "b1rep": t32(rep4(bias1)),            # [128, 1]
        "bz0rep": t32(rep4(const0)),          # [128, 1]
        "bz1rep": t32(rep4(const2)),          # [128, 1]
        "bf1rep": t32(rep4(bf1f)),            # [64, 1]
        "grep": t32(np.tile(Gd.reshape(1, -1), (WT, 1))),   # [128, 256]
        "csumrep": t32(np.full((WT, 1), csum)),             # [128, 1]
    }


_WEIGHT_SHAPES = {
    "u0blk": (G, 128), "p0xblk": (2 * G, 128), "q0xblk": (2 * G, 128),
    "a1blk": (128, 128), "p1blk": (128, 128), "q1blk": (128, 128),
    "m0nblk": (128, 128), "n1a0xblk": (2 * G, 128), "m1nblk": (128, 128),
    "n1azblk": (128, 128), "f1blk": (128, 64), "wf2blk": (64, G),
    "b0rep": (128, 1), "b1rep": (128, 1), "bz0rep": (128, 1),
    "bz1rep": (128, 1), "bf1rep": (64, 1),
    "grep": (WT, N_PART * N_PART), "csumrep": (WT, 1),
}


# ----------------------------------------------------------------------------
# Device program (one core; SPMD across 8)
# ----------------------------------------------------------------------------

def build_bass():
    nc = bacc.Bacc("TRN2", target_bir_lowering=False, debug=False)

    x_d = nc.dram_tensor("x", [B_CORE, N_PART, D], F32, kind="ExternalInput").ap()
    w_d = {
        name: nc.dram_tensor(name, list(shape), F32, kind="ExternalInput").ap()
        for name, shape in _WEIGHT_SHAPES.items()
    }
    out_d = nc.dram_tensor("out", [B_CORE, 1], F32, kind="ExternalOutput").ap()

    with tile.TileContext(nc) as tc:
        _emit(tc, nc, x_d, w_d, out_d)

    nc.compile()
    return nc


def _emit(tc, nc, x_d, w_d, out_d):
    from contextlib import ExitStack
    ctx = ExitStack()
    with ctx:
        consts = ctx.enter_context(tc.tile_pool(name="consts", bufs=1))
        work = ctx.enter_context(tc.tile_pool(name="work", bufs=2))
        hpool = ctx.enter_context(tc.tile_pool(name="hbuf", bufs=2))
        psum_e = ctx.enter_context(tc.tile_pool(name="psum_e", bufs=3, space="PSUM"))
        psum_n = ctx.enter_context(tc.tile_pool(name="psum_n", bufs=2, space="PSUM"))
        psum_r = ctx.enter_context(tc.tile_pool(name="psum_r", bufs=1, space="PSUM"))

        # --- load folded weights once ---
        w = {}
        for name, shape in _WEIGHT_SHAPES.items():
            t = consts.tile(list(shape), F32, tag=name)
            nc.sync.dma_start(out=t[:], in_=w_d[name][:])
            w[name] = t
        eps12 = consts.tile([WT, 1], F32, tag="eps12")
        nc.vector.memset(eps12[:], 1e-12)
        w["eps12"] = eps12

        for t in range(N_TILES):
            _emit_tile(tc, nc, t, x_d, w, out_d, work, hpool, psum_e, psum_n, psum_r)


def _bc_i(node_t, c):
    """Moving-operand AP broadcasting node cols (w,i) over j for chunk c.

    node_t: [P, 512] tile, cols = (w32, i).  Chunk c of the stacked edge
    layout covers w32 in [2c, 2c+2), cols (w32, i, j).
    """
    v = node_t[:].rearrange("p (w i) -> p w i", i=N_PART)
    v = v[:, 2 * c:2 * c + 2, :]
    return v.unsqueeze(3).broadcast_to([v.shape[0], 2, N_PART, N_PART])


def _bc_j(node_t, c):
    """Same, broadcasting node cols (w,j) over i."""
    v = node_t[:].rearrange("p (w i) -> p w i", i=N_PART)
    v = v[:, 2 * c:2 * c + 2, :]
    return v.unsqueeze(2).broadcast_to([v.shape[0], 2, N_PART, N_PART])


def _emit_tile(tc, nc, t, x_d, w, out_d, work, hpool, psum_e, psum_n, psum_r):
    base = t * WT

    # ---- load x for this tile ----
    # [128 partitions = walker, 32 = (i, d)]
    x_t = work.tile([WT, N_PART * D], F32, tag="x_t")
    nc.sync.dma_start(
        out=x_t[:],
        in_=x_d[base:base + WT].rearrange("w i d -> w (i d)"),
    )
    # [8 partitions = (d, g), 512 = (w32, i)] - d-major so each DMA writes
    # a contiguous partition block
    xT = work.tile([2 * G, VCOLS], F32, tag="xT")
    for d in range(D):
        nc.sync.dma_start(
            out=xT[d * G:(d + 1) * G, :],
            in_=x_d[base:base + WT, :, d].rearrange("(g w) i -> g w i", g=G),
        )

    # ---- pair geometry: r1[w, (i,j)] ----
    xv = x_t[:].rearrange("p (i d) -> p i d", d=D)
    dx = work.tile([WT, N_PART * N_PART], F32, tag="dx")
    dy = work.tile([WT, N_PART * N_PART], F32, tag="dy")
    for d, dst in ((0, dx), (1, dy)):
        col = xv[:, :, d]                       # [128, 16]
        a = col.unsqueeze(2).broadcast_to([WT, N_PART, N_PART])
        b = col.unsqueeze(1).broadcast_to([WT, N_PART, N_PART])
        nc.vector.tensor_sub(
            dst[:].rearrange("p (i j) -> p i j", j=N_PART), a, b)
    sq = work.tile([WT, N_PART * N_PART], F32, tag="sq")
    ssq = work.tile([WT, N_PART * N_PART], F32, tag="ssq")
    nc.vector.tensor_mul(sq[:], dx[:], dx[:])
    nc.vector.tensor_mul(ssq[:], dy[:], dy[:])
    nc.vector.tensor_add(ssq[:], ssq[:], sq[:])
    r1 = work.tile([WT, N_PART * N_PART], F32, tag="r1")
    nc.scalar.activation(r1[:], ssq[:], AF.Sqrt, bias=w["eps12"][:])

    # ---- cusp (partition = walker) ----
    rp1 = work.tile([WT, N_PART * N_PART], F32, tag="rp1")
    nc.vector.tensor_scalar_add(rp1[:], r1[:], 1.0)
    inv = work.tile([WT, N_PART * N_PART], F32, tag="inv")
    nc.vector.reciprocal(inv[:], rp1[:])
    prod = work.tile([WT, N_PART * N_PART], F32, tag="prod")
    nc.vector.tensor_mul(prod[:], inv[:], w["grep"][:])
    ssum = work.tile([WT, 1], F32, tag="ssum")
    nc.vector.reduce_sum(ssum[:], prod[:], axis=AX.X)
    # cusp + b_f2 per walker: (sum(Gd) + b_f2) - sum(Gd / (1 + r))
    cusp_pp = work.tile([WT, 1], F32, tag="cusp_pp")
    nc.vector.tensor_sub(cusp_pp[:], w["csumrep"][:], ssum[:])

    # ---- r1 rearranged to stacked layout [4 = g, 8192 = (w32, i, j)] ----
    r1s = work.tile([G, ECOLS], F32, tag="r1s")
    for g in range(G):
        nc.sync.dma_start(
            out=r1s[g:g + 1, :], in_=r1[g * W32:(g + 1) * W32, :])

    # ---- phase 0: h1' = tanh(u0 r1 + (P0x x)_i + (Q0x x)_j + b0) ----
    h1 = hpool.tile([WT, ECOLS], F32, tag="h1")
    for c in range(N_CHUNKS):
        pc = psum_e.tile([WT, CHUNK], F32, tag="pc")
        nc.tensor.matmul(pc[:], w["u0blk"][:], r1s[:, c * CHUNK:(c + 1) * CHUNK],
                         start=True, stop=False)
        nc.tensor.matmul(pc[:], w["p0xblk"][:], _bc_i(xT, c),
                         start=False, stop=False)
        nc.tensor.matmul(pc[:], w["q0xblk"][:], _bc_j(xT, c),
                         start=False, stop=True)
        nc.scalar.activation(h1[:, c * CHUNK:(c + 1) * CHUNK], pc[:],
                             AF.Tanh, bias=w["b0rep"][:])

    # ---- node update 0: z = tanh(M0n s0 + N1a0x x + const0) ----
    s0 = work.tile([WT, VCOLS], F32, tag="s")
    nc.vector.reduce_sum(
        s0[:], h1[:].rearrange("p (wi j) -> p wi j", j=N_PART), axis=AX.X)
    pn = psum_n.tile([WT, VCOLS], F32, tag="pn")
    nc.tensor.matmul(pn[:], w["m0nblk"][:], s0[:], start=True, stop=False)
    nc.tensor.matmul(pn[:], w["n1a0xblk"][:], xT[:], start=False, stop=True)
    z = work.tile([WT, VCOLS], F32, tag="z")
    nc.scalar.activation(z[:], pn[:], AF.Tanh, bias=w["bz0rep"][:])

    # ---- phase 1: h2' = tanh(A1t h1' + (P1 z)_i + (Q1 z)_j + b1), in-place ----
    for c in range(N_CHUNKS):
        pc = psum_e.tile([WT, CHUNK], F32, tag="pc")
        nc.tensor.matmul(pc[:], w["a1blk"][:], h1[:, c * CHUNK:(c + 1) * CHUNK],
                         start=True, stop=False)
        nc.tensor.matmul(pc[:], w["p1blk"][:], _bc_i(z, c),
                         start=False, stop=False)
        nc.tensor.matmul(pc[:], w["q1blk"][:], _bc_j(z, c),
                         start=False, stop=True)
        nc.scalar.activation(h1[:, c * CHUNK:(c + 1) * CHUNK], pc[:],
                             AF.Tanh, bias=w["b1rep"][:])

    # ---- node update 1: z2 = tanh(M1n s1 + N1az z + const2) ----
    s1 = work.tile([WT, VCOLS], F32, tag="s")
    nc.vector.reduce_sum(
        s1[:], h1[:].rearrange("p (wi j) -> p wi j", j=N_PART), axis=AX.X)
    pn2 = psum_n.tile([WT, VCOLS], F32, tag="pn")
    nc.tensor.matmul(pn2[:], w["m1nblk"][:], s1[:], start=True, stop=False)
    nc.tensor.matmul(pn2[:], w["n1azblk"][:], z[:], start=False, stop=True)
    z2 = work.tile([WT, VCOLS], F32, tag="z2")
    nc.scalar.activation(z2[:], pn2[:], AF.Tanh, bias=w["bz1rep"][:])

    # ---- readout ----
    zbar = work.tile([WT, W32], F32, tag="zbar")
    nc.vector.reduce_sum(
        zbar[:], z2[:].rearrange("p (w i) -> p w i", i=N_PART), axis=AX.X)
    pf1 = psum_r.tile([2 * NH, W32], F32, tag="pf1")
    nc.tensor.matmul(pf1[:], w["f1blk"][:], zbar[:], start=True, stop=True)
    f1 = work.tile([2 * NH, W32], F32, tag="f1")
    nc.scalar.activation(f1[:], pf1[:], AF.Tanh, bias=w["bf1rep"][:])
    pf2 = psum_r.tile([G, W32], F32, tag="pf2")
    nc.tensor.matmul(pf2[:], w["wf2blk"][:], f1[:], start=True, stop=True)

    # cusp_pp [128, 1] -> [4, 32] to match (g, w32) layout of pf2
    cusp_r = work.tile([G, W32], F32, tag="cusp_r")
    nc.sync.dma_start(out=cusp_r[:], in_=cusp_pp[:])
    outsb = work.tile([G, W32], F32, tag="outsb")
    nc.vector.tensor_add(outsb[:], pf2[:], cusp_r[:])

    nc.sync.dma_start(
        out=out_d[base:base + WT].rearrange("(g w) one -> g (w one)", g=G),
        in_=outsb[:],
    )


# ----------------------------------------------------------------------------
# Host entry point
# ----------------------------------------------------------------------------

_NC_CACHE = []


def _get_nc():
    if not _NC_CACHE:
        _NC_CACHE.append(build_bass())
    return _NC_CACHE[0]


def kernel(**inputs) -> np.ndarray:
    x = np.ascontiguousarray(np.asarray(inputs["x"], np.float32))
    assert x.shape == (B_FULL, N_PART, D), x.shape
    folded = _fold_params(inputs)

    nc = _get_nc()
    in_maps = []
    for c in range(N_CORES):
        m = {"x": np.ascontiguousarray(x[c * B_CORE:(c + 1) * B_CORE])}
        m.update(folded)
        in_maps.append(m)

    res = run_bass_kernel_spmd(nc, in_maps, list(range(N_CORES)))
    out = np.concatenate([res.results[c]["out"] for c in range(N_CORES)], axis=0)
    return out.astype(np.float32)


# revision 11
# speedup vs baseline: 2.0292x; 2.0292x over previous
"""Trainium2 Bass kernel for nn_CTNNJastrow (GNN message passing Jastrow factor).

Strategy
--------
Pure data parallel: batch dim B=4096 split across 8 NeuronCores (512 walkers
per core).  The tiny MLP weights are replicated (and heavily *folded* on the
host first - see below); no cross-core communication.

Math folding (exact linear algebra, done in float64 on host):
The reference network is, per step, a linear edge-update MLP wrapped in one
tanh, plus a linear node-update MLP wrapped in one tanh.  Every linear op
that is NOT separated from another linear op by a tanh can be fused:

  step0 edge pre-activation  c0  = u0*r1 + (P0x @ x)_i + (Q0x @ x)_j  + b0
  h1' = tanh(c0)                                  (only edge-sized state)
  node pre-act  = M0n @ (sum_j h1') + N1a0x @ x + const0 ;  z = tanh(.)
  step1 edge pre-activation  c1  = A1t @ h1' + (P1 @ z)_i + (Q1 @ z)_j + b1
  h2' = tanh(c1)
  node pre-act  = M1n @ (sum_j h2') + N1az @ z + const2 ;  z2 = tanh(.)
  out = w_f2 @ tanh(F1 @ (sum_i z2) + bf1f) + b_f2 + cusp

so per message-passing step only ONE edge-sized (B*N*N, 32) tanh and three
accumulating matmul streams are needed; eu2 / e2v / v2e / node MLPs all
collapse into 32x32 folded matrices applied at node granularity.

Device layout: walkers are processed in tiles of 128.  Edge tensors use a
"4-group stacked" layout [128 partitions = (group g=w//32, feature f=0..31),
free = (w%32, i, j) = 8192 cols] so all engines run with full 128-partition
utilization.  Folded weight matrices are block-diagonalized (4 copies of the
32x32 block) on the host.  The i/j broadcasts are free-dim step-0 access
patterns read directly by the PE as the moving operand; the r1 rank-1 term
enters through a K=4 matmul.

The electron-electron cusp is computed per-walker (partition = walker,
free = 256 edge pairs) with a fused multiply-reduce; the spin-dependent
gamma matrix is precomputed on host from the runtime `spin` input.
"""

import numpy as np

import concourse.bass as bass
import concourse.tile as tile
from concourse import bacc, mybir
from concourse.bass_utils import run_bass_kernel_spmd

F32 = mybir.dt.float32
BF16 = mybir.dt.bfloat16
AF = mybir.ActivationFunctionType
ALU = mybir.AluOpType
AX = mybir.AxisListType

# Problem constants (fixed by the module definition)
N_PART = 16
D = 2
NH = 32
EH = 32
RH = 16
B_FULL = 4096
N_CORES = 8
B_CORE = B_FULL // N_CORES      # 512 walkers per core
WT = 128                        # walkers per on-chip tile
N_TILES = B_CORE // WT          # 4
G = 4                           # partition stacking groups (of 32 walkers)
W32 = WT // G                   # 32
ECOLS = W32 * N_PART * N_PART   # 8192 stacked edge cols per tile
VCOLS = W32 * N_PART            # 512 stacked node cols per tile
CHUNK = 512                     # psum bank / fp32 moving-operand limit
N_CHUNKS = ECOLS // CHUNK       # 16


# ----------------------------------------------------------------------------
# Host-side weight folding (float64 for accuracy, cast to f32 at the end)
# ----------------------------------------------------------------------------

def _blkdiag(block: np.ndarray) -> np.ndarray:
    """Replicate a [kb, mb] lhsT block on the diagonal 4x -> [4*kb, 4*mb]."""
    kb, mb = block.shape
    out = np.zeros((G * kb, G * mb), np.float64)
    for g in range(G):
        out[g * kb:(g + 1) * kb, g * mb:(g + 1) * mb] = block
    return out


def _blkdiag_dg(block: np.ndarray) -> np.ndarray:
    """lhsT for K-rows laid out as (d, g) [d-major] instead of (g, d).

    block: [D, mb].  Returns [D*4, 4*mb] with out[d*4+g, g*mb:(g+1)*mb] =
    block[d].  Matches an xT tile whose partitions are d*4+g.
    """
    db, mb = block.shape
    out = np.zeros((db * G, G * mb), np.float64)
    for d in range(db):
        for g in range(G):
            out[d * G + g, g * mb:(g + 1) * mb] = block[d]
    return out


def _fold_params(inp: dict) -> dict:
    f = lambda k: np.asarray(inp[k], np.float64)
    w_ne, w_ee = f("w_ne"), f("w_ee")
    w_v2e, w_eu1, b_eu1 = f("w_v2e"), f("w_eu1"), f("b_eu1")
    w_eu2, b_eu2, w_e2v = f("w_eu2"), f("b_eu2"), f("w_e2v")
    w_nu1, b_nu1, w_nu2, b_nu2 = f("w_nu1"), f("b_nu1"), f("w_nu2"), f("b_nu2")
    w_f1, b_f1, w_f2, b_f2 = f("w_f1"), f("b_f1"), f("w_f2"), f("b_f2")
    spin = np.asarray(inp["spin"])

    A0, B0, C0 = w_eu1[0][:, :EH], w_eu1[0][:, EH:2 * EH], w_eu1[0][:, 2 * EH:]
    A1, B1, C1 = w_eu1[1][:, :EH], w_eu1[1][:, EH:2 * EH], w_eu1[1][:, 2 * EH:]
    w_r, w_i, w_j = w_ee[:, 0:1], w_ee[:, 1:1 + NH], w_ee[:, 1 + NH:]

    # step 0 edge pre-activation: c0 = u0*r1 + (P0x x)_i + (Q0x x)_j + b_eu1[0]
    u0 = A0 @ w_r                                   # [EH, 1]
    P0x = (A0 @ w_i + B0 @ w_v2e[0]) @ w_ne         # [EH, D]
    Q0x = (A0 @ w_j + C0 @ w_v2e[0]) @ w_ne         # [EH, D]

    # step 0 node update (on s0 = sum_j h1' and x):
    N1a0, N1b0 = w_nu1[0][:, :NH], w_nu1[0][:, NH:]
    M0n = N1b0 @ (w_e2v[0] @ w_eu2[0])              # [NH, EH]
    N1a0x = N1a0 @ w_ne                             # [NH, D]
    const0 = N1b0 @ (16.0 * w_e2v[0] @ b_eu2[0]) + b_nu1[0]

    # step 1 edge pre-activation: c1 = A1t h1' + (P1 z)_i + (Q1 z)_j + bias1
    A1t = A1 @ w_eu2[0]                             # [EH, EH]
    P1 = B1 @ w_v2e[1] @ w_nu2[0]                   # [EH, NH]
    Q1 = C1 @ w_v2e[1] @ w_nu2[0]                   # [EH, NH]
    bias1 = b_eu1[1] + A1 @ b_eu2[0] + (B1 + C1) @ (w_v2e[1] @ b_nu2[0])

    # step 1 node update (on s1 = sum_j h2' and z):
    N1a1, N1b1 = w_nu1[1][:, :NH], w_nu1[1][:, NH:]
    M1n = N1b1 @ (w_e2v[1] @ w_eu2[1])              # [NH, EH]
    N1az = N1a1 @ w_nu2[0]                          # [NH, NH]
    const2 = N1a1 @ b_nu2[0] + N1b1 @ (16.0 * w_e2v[1] @ b_eu2[1]) + b_nu1[1]

    # readout (on sum_i z2):
    F1 = (w_f1 @ w_nu2[1]) / 16.0                   # [RH, NH]
    bf1f = w_f1 @ b_nu2[1] + b_f1                   # [RH]

    # cusp: gamma over ALL ordered pairs, 0.1 factor (0.2 * triu->full/2),
    # diagonal zeroed. gamma_para = 1/(D+1), gamma_apara = 1/(D-1).
    same = (spin[:, None] == spin[None, :]).astype(np.float64)
    gamma = same * (1.0 / (D + 1)) + (1.0 - same) * (1.0 / (D - 1))
    Gd = 0.1 * gamma
    np.fill_diagonal(Gd, 0.0)
    csum = Gd.sum() + float(b_f2[0])

    rep4 = lambda v: np.tile(np.asarray(v, np.float64), G)[:, None]
    t32 = lambda a: np.ascontiguousarray(a, np.float32)
    import ml_dtypes
    tbf = lambda a: np.ascontiguousarray(a, ml_dtypes.bfloat16)

    return {
        "u0blk": tbf(_blkdiag(u0.T)),         # [4, 128]
        "p0xblk": tbf(_blkdiag_dg(P0x.T)),    # [8, 128] (d,g) rows
        "q0xblk": tbf(_blkdiag_dg(Q0x.T)),    # [8, 128] (d,g) rows
        "a1blk": tbf(_blkdiag(A1t.T)),        # [128, 128]
        "p1blk": tbf(_blkdiag(P1.T)),         # [128, 128]
        "q1blk": tbf(_blkdiag(Q1.T)),         # [128, 128]
        "m0nblk": t32(_blkdiag(M0n.T)),       # [128, 128]
        "n1a0xblk": tbf(_blkdiag_dg(N1a0x.T)),  # [8, 128] (d,g) rows
        "m1nblk": t32(_blkdiag(M1n.T)),       # [128, 128]
        "n1azblk": tbf(_blkdiag(N1az.T)),     # [128, 128]
        "f1blk": t32(_blkdiag(F1.T)),         # [128, 64]
        "wf2blk": t32(_blkdiag(w_f2.T)),      # [64, 4]
        "b0rep": t32(rep4(b_eu1[0])),         # [128, 1]
        "b1rep": t32(rep4(bias1)),            # [128, 1]
        "bz0rep": t32(rep4(const0)),          # [128, 1]
        "bz1rep": t32(rep4(const2)),          # [128, 1]
        "bf1rep": t32(rep4(bf1f)),            # [64, 1]
        "grep": t32(np.tile(Gd.reshape(1, -1), (WT, 1))),   # [128, 256]
        "csumrep": t32(np.full((WT, 1), csum)),             # [128, 1]
    }


_BF_NAMES = {"u0blk", "p0xblk", "q0xblk", "a1blk", "p1blk", "q1blk",
             "n1a0xblk", "n1azblk"}
_WEIGHT_SHAPES = {
    "u0blk": (G, 128), "p0xblk": (2 * G, 128), "q0xblk": (2 * G, 128),
    "a1blk": (128, 128), "p1blk": (128, 128), "q1blk": (128, 128),
    "m0nblk": (128, 128), "n1a0xblk": (2 * G, 128), "m1nblk": (128, 128),
    "n1azblk": (128, 128), "f1blk": (128, 64), "wf2blk": (64, G),
    "b0rep": (128, 1), "b1rep": (128, 1), "bz0rep": (128, 1),
    "bz1rep": (128, 1), "bf1rep": (64, 1),
    "grep": (WT, N_PART * N_PART), "csumrep": (WT, 1),
}


def _wdt(name):
    return BF16 if name in _BF_NAMES else F32


# ----------------------------------------------------------------------------
# Device program (one core; SPMD across 8)
# ----------------------------------------------------------------------------

def build_bass():
    nc = bacc.Bacc("TRN2", target_bir_lowering=False, debug=False)

    x_d = nc.dram_tensor("x", [B_CORE, N_PART, D], F32, kind="ExternalInput").ap()
    xbf_d = nc.dram_tensor(
        "xbf", [B_CORE, N_PART, D], BF16, kind="ExternalInput").ap()
    w_d = {
        name: nc.dram_tensor(name, list(shape), _wdt(name),
                             kind="ExternalInput").ap()
        for name, shape in _WEIGHT_SHAPES.items()
    }
    out_d = nc.dram_tensor("out", [B_CORE, 1], F32, kind="ExternalOutput").ap()

    with tile.TileContext(nc) as tc:
        _emit(tc, nc, x_d, xbf_d, w_d, out_d)

    nc.compile()
    return nc


def _emit(tc, nc, x_d, xbf_d, w_d, out_d):
    from contextlib import ExitStack
    ctx = ExitStack()
    with ctx:
        consts = ctx.enter_context(tc.tile_pool(name="consts", bufs=1))
        work = ctx.enter_context(tc.tile_pool(name="work", bufs=2))
        hpool = ctx.enter_context(tc.tile_pool(name="hbuf", bufs=2))
        psum_e = ctx.enter_context(tc.tile_pool(name="psum_e", bufs=3, space="PSUM"))
        psum_n = ctx.enter_context(tc.tile_pool(name="psum_n", bufs=2, space="PSUM"))
        psum_r = ctx.enter_context(tc.tile_pool(name="psum_r", bufs=1, space="PSUM"))

        # --- load folded weights once ---
        w = {}
        for name, shape in _WEIGHT_SHAPES.items():
            t = consts.tile(list(shape), _wdt(name), tag=name)
            nc.sync.dma_start(out=t[:], in_=w_d[name][:])
            w[name] = t
        eps12 = consts.tile([WT, 1], F32, tag="eps12")
        nc.vector.memset(eps12[:], 1e-12)
        w["eps12"] = eps12

        for t in range(N_TILES):
            _emit_tile(tc, nc, t, x_d, xbf_d, w, out_d, work, hpool, psum_e, psum_n, psum_r)


def _bc_i(node_t, c):
    """Moving-operand AP broadcasting node cols (w,i) over j for chunk c.

    node_t: [P, 512] tile, cols = (w32, i).  Chunk c of the stacked edge
    layout covers w32 in [2c, 2c+2), cols (w32, i, j).
    """
    v = node_t[:].rearrange("p (w i) -> p w i", i=N_PART)
    v = v[:, 2 * c:2 * c + 2, :]
    return v.unsqueeze(3).broadcast_to([v.shape[0], 2, N_PART, N_PART])


def _bc_j(node_t, c):
    """Same, broadcasting node cols (w,j) over i."""
    v = node_t[:].rearrange("p (w i) -> p w i", i=N_PART)
    v = v[:, 2 * c:2 * c + 2, :]
    return v.unsqueeze(2).broadcast_to([v.shape[0], 2, N_PART, N_PART])


def _emit_tile(tc, nc, t, x_d, xbf_d, w, out_d, work, hpool, psum_e, psum_n, psum_r):
    base = t * WT

    # ---- load x for this tile ----
    # [128 partitions = walker, 32 = (i, d)]
    x_t = work.tile([WT, N_PART * D], F32, tag="x_t")
    nc.sync.dma_start(
        out=x_t[:],
        in_=x_d[base:base + WT].rearrange("w i d -> w (i d)"),
    )
    # [8 partitions = (d, g), 512 = (w32, i)] - d-major so each DMA writes
    # a contiguous partition block
    xT = work.tile([2 * G, VCOLS], BF16, tag="xT")
    for d in range(D):
        nc.sync.dma_start(
            out=xT[d * G:(d + 1) * G, :],
            in_=xbf_d[base:base + WT, :, d].rearrange("(g w) i -> g w i", g=G),
        )

    # ---- pair geometry: r1[w, (i,j)] ----
    xv = x_t[:].rearrange("p (i d) -> p i d", d=D)
    dx = work.tile([WT, N_PART * N_PART], F32, tag="dx")
    dy = work.tile([WT, N_PART * N_PART], F32, tag="dy")
    for d, dst in ((0, dx), (1, dy)):
        col = xv[:, :, d]                       # [128, 16]
        a = col.unsqueeze(2).broadcast_to([WT, N_PART, N_PART])
        b = col.unsqueeze(1).broadcast_to([WT, N_PART, N_PART])
        nc.vector.tensor_sub(
            dst[:].rearrange("p (i j) -> p i j", j=N_PART), a, b)
    sq = work.tile([WT, N_PART * N_PART], F32, tag="sq")
    ssq = work.tile([WT, N_PART * N_PART], F32, tag="ssq")
    nc.vector.tensor_mul(sq[:], dx[:], dx[:])
    nc.vector.tensor_mul(ssq[:], dy[:], dy[:])
    nc.vector.tensor_add(ssq[:], ssq[:], sq[:])
    r1 = work.tile([WT, N_PART * N_PART], F32, tag="r1")
    nc.scalar.activation(r1[:], ssq[:], AF.Sqrt, bias=w["eps12"][:])

    # ---- cusp (partition = walker) ----
    rp1 = work.tile([WT, N_PART * N_PART], F32, tag="rp1")
    nc.vector.tensor_scalar_add(rp1[:], r1[:], 1.0)
    inv = work.tile([WT, N_PART * N_PART], F32, tag="inv")
    nc.vector.reciprocal(inv[:], rp1[:])
    prod = work.tile([WT, N_PART * N_PART], F32, tag="prod")
    nc.vector.tensor_mul(prod[:], inv[:], w["grep"][:])
    ssum = work.tile([WT, 1], F32, tag="ssum")
    nc.vector.reduce_sum(ssum[:], prod[:], axis=AX.X)
    # cusp + b_f2 per walker: (sum(Gd) + b_f2) - sum(Gd / (1 + r))
    cusp_pp = work.tile([WT, 1], F32, tag="cusp_pp")
    nc.vector.tensor_sub(cusp_pp[:], w["csumrep"][:], ssum[:])

    # ---- r1 rearranged to stacked layout [4 = g, 8192 = (w32, i, j)] ----
    r1b = work.tile([WT, N_PART * N_PART], BF16, tag="r1b")
    nc.vector.tensor_copy(r1b[:], r1[:])
    r1s = work.tile([G, ECOLS], BF16, tag="r1s")
    for g in range(G):
        nc.sync.dma_start(
            out=r1s[g:g + 1, :], in_=r1b[g * W32:(g + 1) * W32, :])

    # ---- phase 0: h1' = tanh(u0 r1 + (P0x x)_i + (Q0x x)_j + b0) ----
    h1 = hpool.tile([WT, ECOLS], BF16, tag="h1")
    for c in range(N_CHUNKS):
        pc = psum_e.tile([WT, CHUNK], F32, tag="pc")
        nc.tensor.matmul(pc[:], w["u0blk"][:], r1s[:, c * CHUNK:(c + 1) * CHUNK],
                         start=True, stop=False)
        nc.tensor.matmul(pc[:], w["p0xblk"][:], _bc_i(xT, c),
                         start=False, stop=False)
        nc.tensor.matmul(pc[:], w["q0xblk"][:], _bc_j(xT, c),
                         start=False, stop=True)
        nc.scalar.activation(h1[:, c * CHUNK:(c + 1) * CHUNK], pc[:],
                             AF.Tanh, bias=w["b0rep"][:])

    # ---- node update 0: z = tanh(M0n s0 + N1a0x x + const0) ----
    s0 = work.tile([WT, VCOLS], F32, tag="s")
    nc.vector.reduce_sum(
        s0[:], h1[:].rearrange("p (wi j) -> p wi j", j=N_PART), axis=AX.X)
    pn = psum_n.tile([WT, VCOLS], F32, tag="pn")
    nc.tensor.matmul(pn[:], w["m0nblk"][:], s0[:], start=True, stop=False)
    nc.tensor.matmul(pn[:], w["n1a0xblk"][:], xT[:], start=False, stop=True)
    z = work.tile([WT, VCOLS], BF16, tag="z")
    nc.scalar.activation(z[:], pn[:], AF.Tanh, bias=w["bz0rep"][:])

    # ---- phase 1: h2' = tanh(A1t h1' + (P1 z)_i + (Q1 z)_j + b1), in-place ----
    for c in range(N_CHUNKS):
        pc = psum_e.tile([WT, CHUNK], F32, tag="pc")
        nc.tensor.matmul(pc[:], w["a1blk"][:], h1[:, c * CHUNK:(c + 1) * CHUNK],
                         start=True, stop=False)
        nc.tensor.matmul(pc[:], w["p1blk"][:], _bc_i(z, c),
                         start=False, stop=False)
        nc.tensor.matmul(pc[:], w["q1blk"][:], _bc_j(z, c),
                         start=False, stop=True)
        nc.scalar.activation(h1[:, c * CHUNK:(c + 1) * CHUNK], pc[:],
                             AF.Tanh, bias=w["b1rep"][:])

    # ---- node update 1: z2 = tanh(M1n s1 + N1az z + const2) ----
    s1 = work.tile([WT, VCOLS], F32, tag="s")
    nc.vector.reduce_sum(
        s1[:], h1[:].rearrange("p (wi j) -> p wi j", j=N_PART), axis=AX.X)
    pn2 = psum_n.tile([WT, VCOLS], F32, tag="pn")
    nc.tensor.matmul(pn2[:], w["m1nblk"][:], s1[:], start=True, stop=False)
    nc.tensor.matmul(pn2[:], w["n1azblk"][:], z[:], start=False, stop=True)
    z2 = work.tile([WT, VCOLS], F32, tag="z2")
    nc.scalar.activation(z2[:], pn2[:], AF.Tanh, bias=w["bz1rep"][:])

    # ---- readout ----
    zbar = work.tile([WT, W32], F32, tag="zbar")
    nc.vector.reduce_sum(
        zbar[:], z2[:].rearrange("p (w i) -> p w i", i=N_PART), axis=AX.X)
    pf1 = psum_r.tile([2 * NH, W32], F32, tag="pf1")
    nc.tensor.matmul(pf1[:], w["f1blk"][:], zbar[:], start=True, stop=True)
    f1 = work.tile([2 * NH, W32], F32, tag="f1")
    nc.scalar.activation(f1[:], pf1[:], AF.Tanh, bias=w["bf1rep"][:])
    pf2 = psum_r.tile([G, W32], F32, tag="pf2")
    nc.tensor.matmul(pf2[:], w["wf2blk"][:], f1[:], start=True, stop=True)

    # cusp_pp [128, 1] -> [4, 32] to match (g, w32) layout of pf2
    cusp_r = work.tile([G, W32], F32, tag="cusp_r")
    nc.sync.dma_start(out=cusp_r[:], in_=cusp_pp[:])
    outsb = work.tile([G, W32], F32, tag="outsb")
    nc.vector.tensor_add(outsb[:], pf2[:], cusp_r[:])

    nc.sync.dma_start(
        out=out_d[base:base + WT].rearrange("(g w) one -> g (w one)", g=G),
        in_=outsb[:],
    )


# ----------------------------------------------------------------------------
# Host entry point
# ----------------------------------------------------------------------------

_NC_CACHE = []


def _get_nc():
    if not _NC_CACHE:
        _NC_CACHE.append(build_bass())
    return _NC_CACHE[0]


def kernel(**inputs) -> np.ndarray:
    x = np.ascontiguousarray(np.asarray(inputs["x"], np.float32))
    assert x.shape == (B_FULL, N_PART, D), x.shape
    folded = _fold_params(inputs)

    import ml_dtypes

    xbf = np.ascontiguousarray(x.astype(ml_dtypes.bfloat16))
    nc = _get_nc()
    in_maps = []
    for c in range(N_CORES):
        m = {
            "x": np.ascontiguousarray(x[c * B_CORE:(c + 1) * B_CORE]),
            "xbf": np.ascontiguousarray(xbf[c * B_CORE:(c + 1) * B_CORE]),
        }
        m.update(folded)
        in_maps.append(m)

    res = run_bass_kernel_spmd(nc, in_maps, list(range(N_CORES)))
    out = np.concatenate([res.results[c]["out"] for c in range(N_CORES)], axis=0)
    return out.astype(np.float32)


# revision 15
# speedup vs baseline: 2.2042x; 1.0862x over previous
"""Trainium2 Bass kernel for nn_CTNNJastrow (GNN message passing Jastrow factor).

Strategy
--------
Pure data parallel: batch dim B=4096 split across 8 NeuronCores (512 walkers
per core).  The tiny MLP weights are replicated (and heavily *folded* on the
host first - see below); no cross-core communication.

Math folding (exact linear algebra, done in float64 on host):
The reference network is, per step, a linear edge-update MLP wrapped in one
tanh, plus a linear node-update MLP wrapped in one tanh.  Every linear op
that is NOT separated from another linear op by a tanh is fused:

  step0 edge pre-activation  c0  = u0*r1 + (P0x @ x)_i + (Q0x @ x)_j  + b0
  h1' = tanh(c0)                                  (only edge-sized state)
  node pre-act  = M0n @ (sum_j h1') + N1a0x @ x + const0 ;  z = tanh(.)
  step1 edge pre-activation  c1  = A1t @ h1' + (P1 @ z)_i + (Q1 @ z)_j + b1
  h2' = tanh(c1)
  node pre-act  = M1n @ (sum_j h2') + N1az @ z + const2 ;  z2 = tanh(.)
  out = w_f2 @ tanh(F1 @ (sum_i z2) + bf1f) + b_f2 + cusp

so per message-passing step only ONE edge-sized (B*N*N, 32) tanh and three
accumulating matmul streams are needed; eu2 / e2v / v2e / node MLPs all
collapse into 32x32 folded matrices applied at node granularity.

Device layout: walkers are processed in tiles of 128.  Edge tensors use a
"4-group stacked" layout [128 partitions = (group g=w//32, feature f=0..31),
free = (w%32, i, j) = 8192 cols] so all engines run with full 128-partition
utilization.  Folded weight matrices are block-diagonalized (4 copies of the
32x32 block) on the host and streamed as bf16 (the edge contribution to the
output is small vs the fp32 cusp term, so bf16 rounding stays ~1e-5 of the
output).  The i/j broadcasts are free-dim step-0 access patterns read
directly by the PE as the moving operand; the r1 rank-1 term enters through
a K=4 matmul.  PSUM is processed in [128,1024] two-bank super-chunks (two
3-matmul accumulation groups + one tanh), and DMAs are spread over the SP
and ACT hardware DGE queues.

The electron-electron cusp is computed per-walker (partition = walker,
free = 256 ordered pairs) in fp32; the spin-dependent gamma matrix is
precomputed on host from the runtime `spin` input (0.2 * triu == 0.1 * all
ordered off-diagonal pairs by symmetry).
"""

import numpy as np

import concourse.bass as bass
import concourse.tile as tile
from concourse import bacc, mybir
from concourse.bass_utils import run_bass_kernel_spmd

F32 = mybir.dt.float32
BF16 = mybir.dt.bfloat16
AF = mybir.ActivationFunctionType
ALU = mybir.AluOpType
AX = mybir.AxisListType
POOL = mybir.PoolFunctionType

# Problem constants (fixed by the module definition)
N_PART = 16
D = 2
NH = 32
EH = 32
RH = 16
B_FULL = 4096
N_CORES = 8
B_CORE = B_FULL // N_CORES      # 512 walkers per core
WT = 128                        # walkers per on-chip tile
N_TILES = B_CORE // WT          # 4
G = 4                           # partition stacking groups (of 32 walkers)
W32 = WT // G                   # 32
ECOLS = W32 * N_PART * N_PART   # 8192 stacked edge cols per tile
VCOLS = W32 * N_PART            # 512 stacked node cols per tile
CHUNK = 512                     # psum bank / accumulation-group width
SUPER = 1024                    # two-bank psum super-chunk
N_SUPER = ECOLS // SUPER        # 8

# packed weight blob layouts: name -> (rows, cols, col_offset)
_BF_BLOB = {}
_F32_BLOB = {}
for _name, _r, _c in [
    ("u0blk", G, 128), ("p0xblk", 2 * G, 128), ("q0xblk", 2 * G, 128),
    ("a1blk", 128, 128), ("p1blk", 128, 128), ("q1blk", 128, 128),
    ("n1a0xblk", 2 * G, 128), ("n1azblk", 128, 128),
]:
    _BF_BLOB[_name] = (_r, _c, sum(v[1] for v in _BF_BLOB.values()))
for _name, _r, _c in [
    ("m0nblk", 128, 128), ("m1nblk", 128, 128), ("f1blk", 128, 64),
    ("wf2blk", 64, G), ("b0rep", 128, 1), ("b1rep", 128, 1),
    ("bz0rep", 128, 1), ("bz1rep", 128, 1), ("bf1rep", 64, 1),
    ("grep", WT, N_PART * N_PART), ("csumrep", WT, 1),
]:
    _F32_BLOB[_name] = (_r, _c, sum(v[1] for v in _F32_BLOB.values()))
BF_COLS = sum(v[1] for v in _BF_BLOB.values())
F32_COLS = sum(v[1] for v in _F32_BLOB.values())


# ----------------------------------------------------------------------------
# Host-side weight folding (float64 for accuracy, cast at the end)
# ----------------------------------------------------------------------------

def _blkdiag(block: np.ndarray) -> np.ndarray:
    """Replicate a [kb, mb] lhsT block on the diagonal 4x -> [4*kb, 4*mb]."""
    kb, mb = block.shape
    out = np.zeros((G * kb, G * mb), np.float64)
    for g in range(G):
        out[g * kb:(g + 1) * kb, g * mb:(g + 1) * mb] = block
    return out


def _blkdiag_dg(block: np.ndarray) -> np.ndarray:
    """lhsT for K-rows laid out as (d, g) [d-major]: matches the xT tile
    whose partitions are d*4+g (so each of the two per-d DMAs writes a
    contiguous partition block)."""
    db, mb = block.shape
    out = np.zeros((db * G, G * mb), np.float64)
    for d in range(db):
        for g in range(G):
            out[d * G + g, g * mb:(g + 1) * mb] = block[d]
    return out


def _fold_params(inp: dict) -> dict:
    import ml_dtypes

    f = lambda k: np.asarray(inp[k], np.float64)
    w_ne, w_ee = f("w_ne"), f("w_ee")
    w_v2e, w_eu1, b_eu1 = f("w_v2e"), f("w_eu1"), f("b_eu1")
    w_eu2, b_eu2, w_e2v = f("w_eu2"), f("b_eu2"), f("w_e2v")
    w_nu1, b_nu1, w_nu2, b_nu2 = f("w_nu1"), f("b_nu1"), f("w_nu2"), f("b_nu2")
    w_f1, b_f1, w_f2, b_f2 = f("w_f1"), f("b_f1"), f("w_f2"), f("b_f2")
    spin = np.asarray(inp["spin"])

    A0, B0, C0 = w_eu1[0][:, :EH], w_eu1[0][:, EH:2 * EH], w_eu1[0][:, 2 * EH:]
    A1, B1, C1 = w_eu1[1][:, :EH], w_eu1[1][:, EH:2 * EH], w_eu1[1][:, 2 * EH:]
    w_r, w_i, w_j = w_ee[:, 0:1], w_ee[:, 1:1 + NH], w_ee[:, 1 + NH:]

    # step 0 edge pre-activation: c0 = u0*r1 + (P0x x)_i + (Q0x x)_j + b_eu1[0]
    u0 = A0 @ w_r                                   # [EH, 1]
    P0x = (A0 @ w_i + B0 @ w_v2e[0]) @ w_ne         # [EH, D]
    Q0x = (A0 @ w_j + C0 @ w_v2e[0]) @ w_ne         # [EH, D]

    # step 0 node update (on s0 = sum_j h1' and x):
    N1a0, N1b0 = w_nu1[0][:, :NH], w_nu1[0][:, NH:]
    M0n = N1b0 @ (w_e2v[0] @ w_eu2[0])              # [NH, EH]
    N1a0x = N1a0 @ w_ne                             # [NH, D]
    const0 = N1b0 @ (16.0 * w_e2v[0] @ b_eu2[0]) + b_nu1[0]

    # step 1 edge pre-activation: c1 = A1t h1' + (P1 z)_i + (Q1 z)_j + bias1
    A1t = A1 @ w_eu2[0]                             # [EH, EH]
    P1 = B1 @ w_v2e[1] @ w_nu2[0]                   # [EH, NH]
    Q1 = C1 @ w_v2e[1] @ w_nu2[0]                   # [EH, NH]
    bias1 = b_eu1[1] + A1 @ b_eu2[0] + (B1 + C1) @ (w_v2e[1] @ b_nu2[0])

    # step 1 node update (on s1 = sum_j h2' and z):
    N1a1, N1b1 = w_nu1[1][:, :NH], w_nu1[1][:, NH:]
    M1n = N1b1 @ (w_e2v[1] @ w_eu2[1])              # [NH, EH]
    N1az = N1a1 @ w_nu2[0]                          # [NH, NH]
    const2 = N1a1 @ b_nu2[0] + N1b1 @ (16.0 * w_e2v[1] @ b_eu2[1]) + b_nu1[1]

    # readout (on sum_i z2):
    F1 = (w_f1 @ w_nu2[1]) / 16.0                   # [RH, NH]
    bf1f = w_f1 @ b_nu2[1] + b_f1                   # [RH]

    # cusp: gamma over ALL ordered pairs, 0.1 factor (0.2 * triu -> full/2),
    # diagonal zeroed. gamma_para = 1/(D+1), gamma_apara = 1/(D-1).
    same = (spin[:, None] == spin[None, :]).astype(np.float64)
    gamma = same * (1.0 / (D + 1)) + (1.0 - same) * (1.0 / (D - 1))
    Gd = 0.1 * gamma
    np.fill_diagonal(Gd, 0.0)
    csum = Gd.sum() + float(b_f2[0])

    rep4 = lambda v: np.tile(np.asarray(v, np.float64), G)[:, None]
    vals = {
        "u0blk": _blkdiag(u0.T),
        "p0xblk": _blkdiag_dg(P0x.T),
        "q0xblk": _blkdiag_dg(Q0x.T),
        "a1blk": _blkdiag(A1t.T),
        "p1blk": _blkdiag(P1.T),
        "q1blk": _blkdiag(Q1.T),
        "n1a0xblk": _blkdiag_dg(N1a0x.T),
        "n1azblk": _blkdiag(N1az.T),
        "m0nblk": _blkdiag(M0n.T),
        "m1nblk": _blkdiag(M1n.T),
        "f1blk": _blkdiag(F1.T),
        "wf2blk": _blkdiag(w_f2.T),
        "b0rep": rep4(b_eu1[0]),
        "b1rep": rep4(bias1),
        "bz0rep": rep4(const0),
        "bz1rep": rep4(const2),
        "bf1rep": rep4(bf1f),
        "grep": np.tile(Gd.reshape(1, -1), (WT, 1)),
        "csumrep": np.full((WT, 1), csum),
    }
    wbf = np.zeros((WT, BF_COLS), np.float64)
    for name, (r, c, off) in _BF_BLOB.items():
        wbf[:r, off:off + c] = vals[name]
    wf32 = np.zeros((WT, F32_COLS), np.float64)
    for name, (r, c, off) in _F32_BLOB.items():
        wf32[:r, off:off + c] = vals[name]
    return {
        "wbf": np.ascontiguousarray(wbf, ml_dtypes.bfloat16),
        "wf32": np.ascontiguousarray(wf32, np.float32),
    }


# ----------------------------------------------------------------------------
# Device program (one core; SPMD across 8)
# ----------------------------------------------------------------------------

def build_bass():
    nc = bacc.Bacc("TRN2", target_bir_lowering=False, debug=False)

    x_d = nc.dram_tensor("x", [B_CORE, N_PART, D], F32, kind="ExternalInput").ap()
    xbf_d = nc.dram_tensor(
        "xbf", [B_CORE, N_PART, D], BF16, kind="ExternalInput").ap()
    wbf_d = nc.dram_tensor("wbf", [WT, BF_COLS], BF16, kind="ExternalInput").ap()
    wf32_d = nc.dram_tensor("wf32", [WT, F32_COLS], F32, kind="ExternalInput").ap()
    out_d = nc.dram_tensor("out", [B_CORE, 1], F32, kind="ExternalOutput").ap()

    with tile.TileContext(nc) as tc:
        _emit(tc, nc, x_d, xbf_d, wbf_d, wf32_d, out_d)

    nc.compile()
    return nc


def _emit(tc, nc, x_d, xbf_d, wbf_d, wf32_d, out_d):
    from contextlib import ExitStack
    ctx = ExitStack()
    with ctx:
        consts = ctx.enter_context(tc.tile_pool(name="consts", bufs=1))
        work = ctx.enter_context(tc.tile_pool(name="work", bufs=2))
        hpool = ctx.enter_context(tc.tile_pool(name="hbuf", bufs=2))
        psum_e = ctx.enter_context(tc.tile_pool(name="psum_e", bufs=3, space="PSUM"))
        psum_n = ctx.enter_context(tc.tile_pool(name="psum_n", bufs=1, space="PSUM"))
        psum_r = ctx.enter_context(tc.tile_pool(name="psum_r", bufs=1, space="PSUM"))

        # --- load packed weights (2 DMAs) ---
        wbf_t = consts.tile([WT, BF_COLS], BF16, tag="wbf")
        nc.sync.dma_start(out=wbf_t[:], in_=wbf_d[:])
        wf32_t = consts.tile([WT, F32_COLS], F32, tag="wf32")
        nc.scalar.dma_start(out=wf32_t[:], in_=wf32_d[:])
        w = {}
        for name, (r, c, off) in _BF_BLOB.items():
            w[name] = wbf_t[0:r, off:off + c]
        for name, (r, c, off) in _F32_BLOB.items():
            w[name] = wf32_t[0:r, off:off + c]
        eps12 = consts.tile([WT, 1], F32, tag="eps12")
        nc.vector.memset(eps12[:], 1e-12)
        w["eps12"] = eps12[:]

        for t in range(N_TILES):
            _emit_tile(tc, nc, t, x_d, xbf_d, w, out_d,
                       work, hpool, psum_e, psum_n, psum_r)


def _bc_half(node_t, c, is_i):
    """512-col moving operand broadcasting node cols over j (is_i) or i.

    node_t: [P, 512], cols = (w32, i).  Chunk c covers w32 in [2c, 2c+2) of
    the stacked edge layout (w32, i, j)."""
    v = node_t.rearrange("p (w i) -> p w i", i=N_PART)
    v = v[:, 2 * c:2 * c + 2, :]
    if is_i:
        return v.unsqueeze(3).broadcast_to([v.shape[0], 2, N_PART, N_PART])
    return v.unsqueeze(2).broadcast_to([v.shape[0], 2, N_PART, N_PART])


def _edge_phase(nc, psum_e, h1, mm3, tanh_bias):
    """One edge phase: per 1024-col super-chunk, two 3-matmul accumulation
    groups into a 2-bank psum tile, then one tanh -> h1 (bf16)."""
    for s in range(N_SUPER):
        pc = psum_e.tile([WT, SUPER], F32, tag="pc")
        for half in range(2):
            mm3(pc[:, half * CHUNK:(half + 1) * CHUNK], 2 * s + half)
        nc.scalar.activation(h1[:, s * SUPER:(s + 1) * SUPER], pc[:],
                             AF.Tanh, bias=tanh_bias)


def _emit_tile(tc, nc, t, x_d, xbf_d, w, out_d, work, hpool,
               psum_e, psum_n, psum_r):
    base = t * WT

    # ---- load x for this tile ----
    # [128 partitions = walker, 32 = (i, d)]
    x_t = work.tile([WT, N_PART * D], F32, tag="x_t")
    nc.sync.dma_start(
        out=x_t[:],
        in_=x_d[base:base + WT].rearrange("w i d -> w (i d)"),
    )
    # [8 partitions = (d, g), 512 = (w32, i)] - d-major so each DMA writes
    # a contiguous partition block
    xT = work.tile([2 * G, VCOLS], BF16, tag="xT")
    for d in range(D):
        eng = nc.sync if d == 0 else nc.scalar
        eng.dma_start(
            out=xT[d * G:(d + 1) * G, :],
            in_=xbf_d[base:base + WT, :, d].rearrange("(g w) i -> g w i", g=G),
        )

    # ---- pair geometry: r1[w, (i,j)] ----
    xv = x_t[:].rearrange("p (i d) -> p i d", d=D)
    dx = work.tile([WT, N_PART * N_PART], F32, tag="dx")
    dy = work.tile([WT, N_PART * N_PART], F32, tag="dy")
    for d, dst in ((0, dx), (1, dy)):
        col = xv[:, :, d]                       # [128, 16]
        a = col.unsqueeze(2).broadcast_to([WT, N_PART, N_PART])
        b = col.unsqueeze(1).broadcast_to([WT, N_PART, N_PART])
        nc.vector.tensor_sub(
            dst[:].rearrange("p (i j) -> p i j", j=N_PART), a, b)
    sq = work.tile([WT, N_PART * N_PART], F32, tag="sq")
    ssq = work.tile([WT, N_PART * N_PART], F32, tag="ssq")
    nc.vector.tensor_mul(sq[:], dx[:], dx[:])
    nc.vector.tensor_mul(ssq[:], dy[:], dy[:])
    nc.vector.tensor_add(ssq[:], ssq[:], sq[:])
    r1 = work.tile([WT, N_PART * N_PART], F32, tag="r1")
    nc.scalar.activation(r1[:], ssq[:], AF.Sqrt, bias=w["eps12"])

    # ---- cusp (partition = walker) ----
    rp1 = work.tile([WT, N_PART * N_PART], F32, tag="rp1")
    nc.vector.tensor_scalar_add(rp1[:], r1[:], 1.0)
    inv = work.tile([WT, N_PART * N_PART], F32, tag="inv")
    nc.vector.reciprocal(inv[:], rp1[:])
    prod = work.tile([WT, N_PART * N_PART], F32, tag="prod")
    nc.vector.tensor_mul(prod[:], inv[:], w["grep"])
    ssum = work.tile([WT, 1], F32, tag="ssum")
    nc.vector.reduce_sum(ssum[:], prod[:], axis=AX.X)
    # cusp + b_f2 per walker: (sum(Gd) + b_f2) - sum(Gd / (1 + r))
    cusp_pp = work.tile([WT, 1], F32, tag="cusp_pp")
    nc.vector.tensor_sub(cusp_pp[:], w["csumrep"], ssum[:])

    # ---- r1 (bf16) rearranged to stacked layout [4 = g, 8192 = (w,i,j)] ----
    r1b = work.tile([WT, N_PART * N_PART], BF16, tag="r1b")
    nc.vector.tensor_copy(r1b[:], r1[:])
    r1s = work.tile([G, ECOLS], BF16, tag="r1s")
    for g in range(G):
        eng = nc.sync if g % 2 == 0 else nc.scalar
        eng.dma_start(out=r1s[g:g + 1, :], in_=r1b[g * W32:(g + 1) * W32, :])

    # ---- phase 0: h1' = tanh(u0 r1 + (P0x x)_i + (Q0x x)_j + b0) ----
    h1 = hpool.tile([WT, ECOLS], BF16, tag="h1")

    def mm3_p0(dst, c):
        nc.tensor.matmul(dst, w["u0blk"], r1s[:, c * CHUNK:(c + 1) * CHUNK],
                         start=True, stop=False)
        nc.tensor.matmul(dst, w["p0xblk"], _bc_half(xT[:], c, True),
                         start=False, stop=False)
        nc.tensor.matmul(dst, w["q0xblk"], _bc_half(xT[:], c, False),
                         start=False, stop=True)

    _edge_phase(nc, psum_e, h1, mm3_p0, w["b0rep"])

    # ---- node update 0: z = tanh(M0n avg_j(h1') + N1a0x x + const0) ----
    s0 = work.tile([WT, VCOLS], F32, tag="s")
    nc.vector.reduce_sum(
        s0[:], h1[:].rearrange("p (wi j) -> p wi j", j=N_PART), axis=AX.X)
    pn = psum_n.tile([WT, VCOLS], F32, tag="pn")
    nc.tensor.matmul(pn[:], w["m0nblk"], s0[:], start=True, stop=False)
    nc.tensor.matmul(pn[:], w["n1a0xblk"], xT[:], start=False, stop=True)
    z = work.tile([WT, VCOLS], BF16, tag="z")
    nc.scalar.activation(z[:], pn[:], AF.Tanh, bias=w["bz0rep"])

    # ---- phase 1: h2' = tanh(A1t h1' + (P1 z)_i + (Q1 z)_j + b1), in-place --
    def mm3_p1(dst, c):
        nc.tensor.matmul(dst, w["a1blk"], h1[:, c * CHUNK:(c + 1) * CHUNK],
                         start=True, stop=False)
        nc.tensor.matmul(dst, w["p1blk"], _bc_half(z[:], c, True),
                         start=False, stop=False)
        nc.tensor.matmul(dst, w["q1blk"], _bc_half(z[:], c, False),
                         start=False, stop=True)

    _edge_phase(nc, psum_e, h1, mm3_p1, w["b1rep"])

    # ---- node update 1: z2 = tanh(M1n avg_j(h2') + N1az z + const2) ----
    s1 = work.tile([WT, VCOLS], F32, tag="s")
    nc.vector.reduce_sum(
        s1[:], h1[:].rearrange("p (wi j) -> p wi j", j=N_PART), axis=AX.X)
    pn2 = psum_n.tile([WT, VCOLS], F32, tag="pn")
    nc.tensor.matmul(pn2[:], w["m1nblk"], s1[:], start=True, stop=False)
    nc.tensor.matmul(pn2[:], w["n1azblk"], z[:], start=False, stop=True)
    z2 = work.tile([WT, VCOLS], F32, tag="z2")
    nc.scalar.activation(z2[:], pn2[:], AF.Tanh, bias=w["bz1rep"])

    # ---- readout ----
    zbar = work.tile([WT, W32], F32, tag="zbar")
    nc.vector.reduce_sum(
        zbar[:], z2[:].rearrange("p (w i) -> p w i", i=N_PART), axis=AX.X)
    pf1 = psum_r.tile([2 * NH, W32], F32, tag="pro")
    nc.tensor.matmul(pf1[:], w["f1blk"], zbar[:], start=True, stop=True)
    f1 = work.tile([2 * NH, W32], F32, tag="f1")
    nc.scalar.activation(f1[:], pf1[:], AF.Tanh, bias=w["bf1rep"])
    pf2 = psum_r.tile([G, W32], F32, tag="pro")
    nc.tensor.matmul(pf2[:], w["wf2blk"], f1[:], start=True, stop=True)

    # cusp_pp [128, 1] -> [4, 32] to match (g, w32) layout of pf2
    cusp_r = work.tile([G, W32], F32, tag="cusp_r")
    nc.gpsimd.dma_start(out=cusp_r[:], in_=cusp_pp[:])
    outsb = work.tile([G, W32], F32, tag="outsb")
    nc.vector.tensor_add(outsb[:], pf2[:], cusp_r[:])

    nc.gpsimd.dma_start(
        out=out_d[base:base + WT].rearrange("(g w) one -> g (w one)", g=G),
        in_=outsb[:],
    )


# ----------------------------------------------------------------------------
# Host entry point
# ----------------------------------------------------------------------------

_NC_CACHE = []


def _get_nc():
    if not _NC_CACHE:
        _NC_CACHE.append(build_bass())
    return _NC_CACHE[0]


def kernel(**inputs) -> np.ndarray:
    import ml_dtypes

    x = np.ascontiguousarray(np.asarray(inputs["x"], np.float32))
    assert x.shape == (B_FULL, N_PART, D), x.shape
    folded = _fold_params(inputs)
    xbf = np.ascontiguousarray(x.astype(ml_dtypes.bfloat16))

    nc = _get_nc()
    in_maps = []
    for c in range(N_CORES):
        sl = slice(c * B_CORE, (c + 1) * B_CORE)
        m = {
            "x": np.ascontiguousarray(x[sl]),
            "xbf": np.ascontiguousarray(xbf[sl]),
        }
        m.update(folded)
        in_maps.append(m)

    res = run_bass_kernel_spmd(nc, in_maps, list(range(N_CORES)))
    out = np.concatenate([res.results[c]["out"] for c in range(N_CORES)], axis=0)
    return out.astype(np.float32)


# revision 18
# speedup vs baseline: 3.5639x; 1.6169x over previous
"""Trainium2 Bass kernel for nn_CTNNJastrow (GNN message passing Jastrow factor).

Strategy
--------
Pure data parallel: batch dim B=4096 split across 8 NeuronCores (512 walkers
per core).  The tiny MLP weights are replicated (and heavily *folded* on the
host first - see below); no cross-core communication.

Math folding (exact linear algebra, done in float64 on host):
The reference network is, per step, a linear edge-update MLP wrapped in one
tanh, plus a linear node-update MLP wrapped in one tanh.  Every linear op
that is NOT separated from another linear op by a tanh is fused:

  step0 edge pre-activation  c0  = u0*r1 + (P0x @ x)_i + (Q0x @ x)_j  + b0
  h1' = tanh(c0)                                  (only edge-sized state)
  node pre-act  = M0n @ (sum_j h1') + N1a0x @ x + const0 ;  z = tanh(.)
  step1 edge pre-activation  c1  = A1t @ h1' + (P1 @ z)_i + (Q1 @ z)_j + b1
  h2' = tanh(c1)
  node pre-act  = M1n @ (sum_j h2') + N1az @ z + const2 ;  z2 = tanh(.)
  out = w_f2 @ tanh(F1 @ (sum_i z2) + bf1f) + b_f2 + cusp

so per message-passing step only ONE edge-sized (B*N*N, 32) tanh and three
accumulating matmul streams are needed; eu2 / e2v / v2e / node MLPs all
collapse into 32x32 folded matrices applied at node granularity.

Device layout: walkers are processed in tiles of 128.  Edge tensors use a
"4-group stacked" layout [128 partitions = (group g=w//32, feature f=0..31),
free = (w%32, i, j) = 8192 cols] so all engines run with full 128-partition
utilization.  Folded weight matrices are block-diagonalized (4 copies of the
32x32 block) on the host and streamed as bf16 (the edge contribution to the
output is small vs the fp32 cusp term, so bf16 rounding stays ~1e-5 of the
output).  The i/j broadcasts are free-dim step-0 access patterns read
directly by the PE as the moving operand; the r1 rank-1 term enters through
a K=4 matmul.

Schedule: emission is PHASE-MAJOR across the four walker tiles (loads+
geometry for all tiles, then edge phase 0 for all tiles, node update 0,
edge phase 1, node update 1 + readout).  The per-engine program order this
produces keeps the PE stream dense - while tile t waits on its j-reduction,
tile t+1's matmuls run - which also keeps the PE HAM warm.  The j-reduction
itself is chunked per 1024-col super-chunk so it overlaps the remaining
matmuls.  PSUM is processed in [128,1024] two-bank super-chunks (two
3-matmul accumulation groups + one tanh).  DMAs are spread across the SP /
ACT hardware DGE queues and the SWDGE queues (per-queue DMA bandwidth is the
limit), and the per-(i,j) gamma matrix is expanded on-chip from a single
1KB row via a K=1 broadcast matmul instead of DMAing 128 replicated copies.

The electron-electron cusp is computed per-walker (partition = walker,
free = 256 ordered pairs) in fp32; the spin-dependent gamma matrix is
precomputed on host from the runtime `spin` input (0.2 * triu == 0.1 * all
ordered off-diagonal pairs by symmetry).
"""

import numpy as np

import concourse.bass as bass
import concourse.tile as tile
from concourse import bacc, mybir
from concourse.bass_utils import run_bass_kernel_spmd

F32 = mybir.dt.float32
BF16 = mybir.dt.bfloat16
AF = mybir.ActivationFunctionType
ALU = mybir.AluOpType
AX = mybir.AxisListType

# Problem constants (fixed by the module definition)
N_PART = 16
D = 2
NH = 32
EH = 32
RH = 16
B_FULL = 4096
N_CORES = 8
B_CORE = B_FULL // N_CORES      # 512 walkers per core
WT = 128                        # walkers per on-chip tile
N_TILES = B_CORE // WT          # 4
G = 4                           # partition stacking groups (of 32 walkers)
W32 = WT // G                   # 32
ECOLS = W32 * N_PART * N_PART   # 8192 stacked edge cols per tile
VCOLS = W32 * N_PART            # 512 stacked node cols per tile
CHUNK = 512                     # psum bank / accumulation-group width
SUPER = 1024                    # two-bank psum super-chunk
N_SUPER = ECOLS // SUPER        # 8
NPAIR = N_PART * N_PART         # 256

# packed weight blobs: blob -> {name: (rows, cols, col_offset)}
_WBF0 = {"u0blk": (G, 128, 0), "p0xblk": (2 * G, 128, 128),
         "q0xblk": (2 * G, 128, 256), "n1a0xblk": (2 * G, 128, 384)}
_WBF1 = {"a1blk": (128, 128, 0), "p1blk": (128, 128, 128),
         "q1blk": (128, 128, 256), "n1azblk": (128, 128, 384)}
_WF32M = {"m0nblk": (128, 128, 0), "m1nblk": (128, 128, 128)}
_WF32R = {"f1blk": (128, 64, 0), "wf2blk": (64, G, 64),
          "b0rep": (128, 1, 68), "b1rep": (128, 1, 69),
          "bz0rep": (128, 1, 70), "bz1rep": (128, 1, 71),
          "bf1rep": (64, 1, 72)}


# ----------------------------------------------------------------------------
# Host-side weight folding (float64 for accuracy, cast at the end)
# ----------------------------------------------------------------------------

def _blkdiag(block: np.ndarray) -> np.ndarray:
    """Replicate a [kb, mb] lhsT block on the diagonal 4x -> [4*kb, 4*mb]."""
    kb, mb = block.shape
    out = np.zeros((G * kb, G * mb), np.float64)
    for g in range(G):
        out[g * kb:(g + 1) * kb, g * mb:(g + 1) * mb] = block
    return out


def _blkdiag_dg(block: np.ndarray) -> np.ndarray:
    """lhsT for K-rows laid out as (d, g) [d-major]: matches the xT tile
    whose partitions are d*4+g (so each per-d DMA writes a contiguous
    partition block)."""
    db, mb = block.shape
    out = np.zeros((db * G, G * mb), np.float64)
    for d in range(db):
        for g in range(G):
            out[d * G + g, g * mb:(g + 1) * mb] = block[d]
    return out


def _fold_params(inp: dict) -> dict:
    import ml_dtypes

    f = lambda k: np.asarray(inp[k], np.float64)
    w_ne, w_ee = f("w_ne"), f("w_ee")
    w_v2e, w_eu1, b_eu1 = f("w_v2e"), f("w_eu1"), f("b_eu1")
    w_eu2, b_eu2, w_e2v = f("w_eu2"), f("b_eu2"), f("w_e2v")
    w_nu1, b_nu1, w_nu2, b_nu2 = f("w_nu1"), f("b_nu1"), f("w_nu2"), f("b_nu2")
    w_f1, b_f1, w_f2, b_f2 = f("w_f1"), f("b_f1"), f("w_f2"), f("b_f2")
    spin = np.asarray(inp["spin"])

    A0, B0, C0 = w_eu1[0][:, :EH], w_eu1[0][:, EH:2 * EH], w_eu1[0][:, 2 * EH:]
    A1, B1, C1 = w_eu1[1][:, :EH], w_eu1[1][:, EH:2 * EH], w_eu1[1][:, 2 * EH:]
    w_r, w_i, w_j = w_ee[:, 0:1], w_ee[:, 1:1 + NH], w_ee[:, 1 + NH:]

    # step 0 edge pre-activation: c0 = u0*r1 + (P0x x)_i + (Q0x x)_j + b_eu1[0]
    u0 = A0 @ w_r                                   # [EH, 1]
    P0x = (A0 @ w_i + B0 @ w_v2e[0]) @ w_ne         # [EH, D]
    Q0x = (A0 @ w_j + C0 @ w_v2e[0]) @ w_ne         # [EH, D]

    # step 0 node update (on s0 = sum_j h1' and x):
    N1a0, N1b0 = w_nu1[0][:, :NH], w_nu1[0][:, NH:]
    M0n = N1b0 @ (w_e2v[0] @ w_eu2[0])              # [NH, EH]
    N1a0x = N1a0 @ w_ne                             # [NH, D]
    const0 = N1b0 @ (16.0 * w_e2v[0] @ b_eu2[0]) + b_nu1[0]

    # step 1 edge pre-activation: c1 = A1t h1' + (P1 z)_i + (Q1 z)_j + bias1
    A1t = A1 @ w_eu2[0]                             # [EH, EH]
    P1 = B1 @ w_v2e[1] @ w_nu2[0]                   # [EH, NH]
    Q1 = C1 @ w_v2e[1] @ w_nu2[0]                   # [EH, NH]
    bias1 = b_eu1[1] + A1 @ b_eu2[0] + (B1 + C1) @ (w_v2e[1] @ b_nu2[0])

    # step 1 node update (on s1 = sum_j h2' and z):
    N1a1, N1b1 = w_nu1[1][:, :NH], w_nu1[1][:, NH:]
    M1n = N1b1 @ (w_e2v[1] @ w_eu2[1])              # [NH, EH]
    N1az = N1a1 @ w_nu2[0]                          # [NH, NH]
    const2 = N1a1 @ b_nu2[0] + N1b1 @ (16.0 * w_e2v[1] @ b_eu2[1]) + b_nu1[1]

    # readout (on sum_i z2):
    F1 = (w_f1 @ w_nu2[1]) / 16.0                   # [RH, NH]
    bf1f = w_f1 @ b_nu2[1] + b_f1                   # [RH]

    # cusp: gamma over ALL ordered pairs, 0.1 factor (0.2 * triu -> full/2),
    # diagonal zeroed. gamma_para = 1/(D+1), gamma_apara = 1/(D-1).
    same = (spin[:, None] == spin[None, :]).astype(np.float64)
    gamma = same * (1.0 / (D + 1)) + (1.0 - same) * (1.0 / (D - 1))
    Gd = 0.1 * gamma
    np.fill_diagonal(Gd, 0.0)
    csum = Gd.sum() + float(b_f2[0])

    rep4 = lambda v: np.tile(np.asarray(v, np.float64), G)[:, None]
    vals = {
        "u0blk": _blkdiag(u0.T),
        "p0xblk": _blkdiag_dg(P0x.T),
        "q0xblk": _blkdiag_dg(Q0x.T),
        "n1a0xblk": _blkdiag_dg(N1a0x.T),
        "a1blk": _blkdiag(A1t.T),
        "p1blk": _blkdiag(P1.T),
        "q1blk": _blkdiag(Q1.T),
        "n1azblk": _blkdiag(N1az.T),
        "m0nblk": _blkdiag(M0n.T),
        "m1nblk": _blkdiag(M1n.T),
        "f1blk": _blkdiag(F1.T),
        "wf2blk": _blkdiag(w_f2.T),
        "b0rep": rep4(b_eu1[0]),
        "b1rep": rep4(bias1),
        "bz0rep": rep4(const0),
        "bz1rep": rep4(const2),
        "bf1rep": rep4(bf1f),
    }

    def pack(layout, rows, dtype):
        cols = max(c + o for _, c, o in layout.values())
        blob = np.zeros((rows, cols), np.float64)
        for name, (r, c, off) in layout.items():
            blob[:r, off:off + c] = vals[name]
        return np.ascontiguousarray(blob, dtype)

    gdrow = np.zeros((1, NPAIR + 1), np.float64)
    gdrow[0, :NPAIR] = Gd.reshape(-1)
    gdrow[0, NPAIR] = csum
    return {
        "wbf0": pack(_WBF0, 2 * G, ml_dtypes.bfloat16),
        "wbf1": pack(_WBF1, WT, ml_dtypes.bfloat16),
        "wf32m": pack(_WF32M, WT, np.float32),
        "wf32r": pack(_WF32R, WT, np.float32),
        "gdrow": np.ascontiguousarray(gdrow, np.float32),
    }


# ----------------------------------------------------------------------------
# Device program (one core; SPMD across 8)
# ----------------------------------------------------------------------------

def build_bass():
    nc = bacc.Bacc("TRN2", target_bir_lowering=False, debug=False)

    x_d = nc.dram_tensor("x", [B_CORE, N_PART, D], F32, kind="ExternalInput").ap()
    # xbf is d-major so each per-d slice is contiguous for the xT load
    xbf_d = nc.dram_tensor(
        "xbf", [D, B_CORE, N_PART], BF16, kind="ExternalInput").ap()
    blob_d = {}
    for name, shape, dt in [
        ("wbf0", [2 * G, 512], BF16), ("wbf1", [WT, 512], BF16),
        ("wf32m", [WT, 256], F32), ("wf32r", [WT, 73], F32),
        ("gdrow", [1, NPAIR + 1], F32),
    ]:
        blob_d[name] = nc.dram_tensor(name, shape, dt, kind="ExternalInput").ap()
    out_d = nc.dram_tensor("out", [B_CORE, 1], F32, kind="ExternalOutput").ap()

    with tile.TileContext(nc) as tc:
        _emit(tc, nc, x_d, xbf_d, blob_d, out_d)

    nc.compile()
    return nc


def _bc_half(node_t, c, is_i):
    """512-col moving operand broadcasting node cols over j (is_i) or i.

    node_t: [P, 512], cols = (w32, i).  Chunk c covers w32 in [2c, 2c+2) of
    the stacked edge layout (w32, i, j)."""
    v = node_t.rearrange("p (w i) -> p w i", i=N_PART)
    v = v[:, 2 * c:2 * c + 2, :]
    if is_i:
        return v.unsqueeze(3).broadcast_to([v.shape[0], 2, N_PART, N_PART])
    return v.unsqueeze(2).broadcast_to([v.shape[0], 2, N_PART, N_PART])


def _edge_phase(nc, psum_e, h1, s_out, mm3, tanh_bias):
    """One edge phase for one tile: per 1024-col super-chunk, two 3-matmul
    accumulation groups into a 2-bank psum tile, one tanh -> h1 (bf16), and
    one chunked j-reduction into s_out so the reduce overlaps later chunks."""
    for s in range(N_SUPER):
        pc = psum_e.tile([WT, SUPER], F32, tag="pc")
        for half in range(2):
            mm3(pc[:, half * CHUNK:(half + 1) * CHUNK], 2 * s + half)
        h_slice = h1[:, s * SUPER:(s + 1) * SUPER]
        nc.scalar.activation(h_slice, pc[:], AF.Tanh, bias=tanh_bias)
        nc.vector.reduce_sum(
            s_out[:, s * (SUPER // N_PART):(s + 1) * (SUPER // N_PART)],
            h_slice.rearrange("p (wi j) -> p wi j", j=N_PART), axis=AX.X)


def _emit(tc, nc, x_d, xbf_d, blob_d, out_d):
    from contextlib import ExitStack
    ctx = ExitStack()
    with ctx:
        consts = ctx.enter_context(tc.tile_pool(name="consts", bufs=1))
        work = ctx.enter_context(tc.tile_pool(name="work", bufs=2))
        per4 = ctx.enter_context(tc.tile_pool(name="per4", bufs=4))
        psum_e = ctx.enter_context(tc.tile_pool(name="psum_e", bufs=2, space="PSUM"))
        psum_n = ctx.enter_context(tc.tile_pool(name="psum_n", bufs=2, space="PSUM"))
        psum_r = ctx.enter_context(tc.tile_pool(name="psum_r", bufs=2, space="PSUM"))

        # --- weight blobs, spread across DMA queues ---
        wbf0_t = consts.tile([2 * G, 512], BF16, tag="wbf0")
        nc.sync.dma_start(out=wbf0_t[:], in_=blob_d["wbf0"][:])
        wf32r_t = consts.tile([WT, 73], F32, tag="wf32r")
        nc.scalar.dma_start(out=wf32r_t[:], in_=blob_d["wf32r"][:])
        gdrow_t = consts.tile([1, NPAIR + 1], F32, tag="gdrow")
        nc.scalar.dma_start(out=gdrow_t[:], in_=blob_d["gdrow"][:])
        wbf1_t = consts.tile([WT, 512], BF16, tag="wbf1")
        nc.gpsimd.dma_start(out=wbf1_t[:], in_=blob_d["wbf1"][:])
        wf32m_t = consts.tile([WT, 256], F32, tag="wf32m")
        nc.gpsimd.dma_start(out=wf32m_t[:], in_=blob_d["wf32m"][:])

        w = {}
        for blob_t, layout in ((wbf0_t, _WBF0), (wbf1_t, _WBF1),
                               (wf32m_t, _WF32M), (wf32r_t, _WF32R)):
            for name, (r, c, off) in layout.items():
                w[name] = blob_t[0:r, off:off + c]
        eps12 = consts.tile([WT, 1], F32, tag="eps12")
        nc.vector.memset(eps12[:], 1e-12)
        w["eps12"] = eps12[:]

        # expand gamma row to all 128 partitions via a K=1 broadcast matmul
        ones1 = consts.tile([1, WT], F32, tag="ones1")
        nc.vector.memset(ones1[:], 1.0)
        pg = psum_r.tile([WT, NPAIR + 1], F32, tag="pro")
        nc.tensor.matmul(pg[:], ones1[:], gdrow_t[:], start=True, stop=True)
        grepcs = consts.tile([WT, NPAIR + 1], F32, tag="grepcs")
        nc.scalar.copy(grepcs[:], pg[:])
        w["grep"] = grepcs[:, :NPAIR]
        w["csumrep"] = grepcs[:, NPAIR:NPAIR + 1]

        st = [dict() for _ in range(N_TILES)]
        for t in range(N_TILES):
            _pass_load_geom(nc, t, st[t], x_d, xbf_d, w, work, per4)
        for t in range(N_TILES):
            _pass_phase0(nc, st[t], w, per4, psum_e)
        for t in range(N_TILES):
            _pass_node0(nc, st[t], w, per4, psum_n)
        for t in range(N_TILES):
            _pass_phase1(nc, st[t], w, psum_e)
        for t in range(N_TILES):
            _pass_node1_readout(nc, t, st[t], w, out_d, work, psum_n, psum_r)


def _pass_load_geom(nc, t, s, x_d, xbf_d, w, work, per4):
    base = t * WT
    # [128 partitions = walker, 32 = (i, d)]
    x_t = work.tile([WT, N_PART * D], F32, tag="x_t")
    nc.sync.dma_start(
        out=x_t[:], in_=x_d[base:base + WT].rearrange("w i d -> w (i d)"))
    # [8 partitions = (d, g), 512 = (w32, i)]; contiguous src per (d, g)
    xT = per4.tile([2 * G, VCOLS], BF16, tag="xT")
    for d in range(D):
        nc.scalar.dma_start(
            out=xT[d * G:(d + 1) * G, :],
            in_=xbf_d[d, base:base + WT].rearrange("(g w) i -> g w i", g=G))
    s["xT"] = xT

    # pair geometry: r1[w, (i,j)]
    xv = x_t[:].rearrange("p (i d) -> p i d", d=D)
    dx = work.tile([WT, NPAIR], F32, tag="dx")
    dy = work.tile([WT, NPAIR], F32, tag="dy")
    for d, dst in ((0, dx), (1, dy)):
        col = xv[:, :, d]
        a = col.unsqueeze(2).broadcast_to([WT, N_PART, N_PART])
        b = col.unsqueeze(1).broadcast_to([WT, N_PART, N_PART])
        nc.vector.tensor_sub(
            dst[:].rearrange("p (i j) -> p i j", j=N_PART), a, b)
    sq = work.tile([WT, NPAIR], F32, tag="sq")
    ssq = work.tile([WT, NPAIR], F32, tag="ssq")
    nc.vector.tensor_mul(sq[:], dx[:], dx[:])
    nc.vector.tensor_mul(ssq[:], dy[:], dy[:])
    nc.vector.tensor_add(ssq[:], ssq[:], sq[:])
    r1 = work.tile([WT, NPAIR], F32, tag="r1")
    nc.scalar.activation(r1[:], ssq[:], AF.Sqrt, bias=w["eps12"])

    # cusp chain (fp32, partition = walker)
    rp1 = work.tile([WT, NPAIR], F32, tag="rp1")
    nc.vector.tensor_scalar_add(rp1[:], r1[:], 1.0)
    inv = work.tile([WT, NPAIR], F32, tag="inv")
    nc.vector.reciprocal(inv[:], rp1[:])
    prod = work.tile([WT, NPAIR], F32, tag="prod")
    nc.vector.tensor_mul(prod[:], inv[:], w["grep"])
    ssum = work.tile([WT, 1], F32, tag="ssum")
    nc.vector.reduce_sum(ssum[:], prod[:], axis=AX.X)
    cusp_pp = per4.tile([WT, 1], F32, tag="cusp_pp")
    nc.vector.tensor_sub(cusp_pp[:], w["csumrep"], ssum[:])
    s["cusp_pp"] = cusp_pp

    # r1 (bf16) rearranged to the stacked layout [4 = g, 8192 = (w,i,j)]
    r1b = work.tile([WT, NPAIR], BF16, tag="r1b")
    nc.vector.tensor_copy(r1b[:], r1[:])
    r1s = per4.tile([G, ECOLS], BF16, tag="r1s")
    engs = [nc.gpsimd, nc.gpsimd, nc.sync, nc.scalar]
    for g in range(G):
        engs[g].dma_start(out=r1s[g:g + 1, :], in_=r1b[g * W32:(g + 1) * W32, :])
    s["r1s"] = r1s


def _pass_phase0(nc, s, w, per4, psum_e):
    h1 = per4.tile([WT, ECOLS], BF16, tag="h1")
    s0 = per4.tile([WT, VCOLS], F32, tag="s0")
    xT, r1s = s["xT"], s["r1s"]

    def mm3_p0(dst, c):
        nc.tensor.matmul(dst, w["p0xblk"], _bc_half(xT[:], c, True),
                         start=True, stop=False)
        nc.tensor.matmul(dst, w["q0xblk"], _bc_half(xT[:], c, False),
                         start=False, stop=False)
        nc.tensor.matmul(dst, w["u0blk"], r1s[:, c * CHUNK:(c + 1) * CHUNK],
                         start=False, stop=True)

    _edge_phase(nc, psum_e, h1, s0, mm3_p0, w["b0rep"])
    s["h1"], s["s0"] = h1, s0


def _pass_node0(nc, s, w, per4, psum_n):
    pn = psum_n.tile([WT, VCOLS], F32, tag="pn")
    nc.tensor.matmul(pn[:], w["m0nblk"], s["s0"][:], start=True, stop=False)
    nc.tensor.matmul(pn[:], w["n1a0xblk"], s["xT"][:], start=False, stop=True)
    z = per4.tile([WT, VCOLS], BF16, tag="z")
    nc.scalar.activation(z[:], pn[:], AF.Tanh, bias=w["bz0rep"])
    s["z"] = z


def _pass_phase1(nc, s, w, psum_e):
    h1, z = s["h1"], s["z"]
    s1 = s["s0"]  # reuse the s0 tile; its value was consumed in node0
    del s["s0"]

    def mm3_p1(dst, c):
        nc.tensor.matmul(dst, w["a1blk"], h1[:, c * CHUNK:(c + 1) * CHUNK],
                         start=True, stop=False)
        nc.tensor.matmul(dst, w["p1blk"], _bc_half(z[:], c, True),
                         start=False, stop=False)
        nc.tensor.matmul(dst, w["q1blk"], _bc_half(z[:], c, False),
                         start=False, stop=True)

    _edge_phase(nc, psum_e, h1, s1, mm3_p1, w["b1rep"])
    s["s1"] = s1


def _pass_node1_readout(nc, t, s, w, out_d, work, psum_n, psum_r):
    base = t * WT
    pn2 = psum_n.tile([WT, VCOLS], F32, tag="pn")
    nc.tensor.matmul(pn2[:], w["m1nblk"], s["s1"][:], start=True, stop=False)
    nc.tensor.matmul(pn2[:], w["n1azblk"], s["z"][:], start=False, stop=True)
    z2 = work.tile([WT, VCOLS], F32, tag="z2")
    nc.scalar.activation(z2[:], pn2[:], AF.Tanh, bias=w["bz1rep"])

    zbar = work.tile([WT, W32], F32, tag="zbar")
    nc.vector.reduce_sum(
        zbar[:], z2[:].rearrange("p (w i) -> p w i", i=N_PART), axis=AX.X)
    pf1 = psum_r.tile([2 * NH, W32], F32, tag="pro")
    nc.tensor.matmul(pf1[:], w["f1blk"], zbar[:], start=True, stop=True)
    f1 = work.tile([2 * NH, W32], F32, tag="f1")
    nc.scalar.activation(f1[:], pf1[:], AF.Tanh, bias=w["bf1rep"])
    pf2 = psum_r.tile([G, W32], F32, tag="pro")
    nc.tensor.matmul(pf2[:], w["wf2blk"], f1[:], start=True, stop=True)

    # cusp_pp [128, 1] -> [4, 32] to match the (g, w32) layout of pf2
    cusp_r = work.tile([G, W32], F32, tag="cusp_r")
    nc.gpsimd.dma_start(out=cusp_r[:], in_=s["cusp_pp"][:])
    outsb = work.tile([G, W32], F32, tag="outsb")
    nc.vector.tensor_add(outsb[:], pf2[:], cusp_r[:])

    nc.gpsimd.dma_start(
        out=out_d[base:base + WT].rearrange("(g w) one -> g (w one)", g=G),
        in_=outsb[:])


# ----------------------------------------------------------------------------
# Host entry point
# ----------------------------------------------------------------------------

_NC_CACHE = []


def _get_nc():
    if not _NC_CACHE:
        _NC_CACHE.append(build_bass())
    return _NC_CACHE[0]


def make_in_maps(inputs):
    import ml_dtypes

    x = np.ascontiguousarray(np.asarray(inputs["x"], np.float32))
    assert x.shape == (B_FULL, N_PART, D), x.shape
    folded = _fold_params(inputs)
    # d-major bf16 copy of x: [D, B, N_PART]
    xbf = np.ascontiguousarray(
        x.transpose(2, 0, 1).astype(ml_dtypes.bfloat16))
    in_maps = []
    for c in range(N_CORES):
        sl = slice(c * B_CORE, (c + 1) * B_CORE)
        m = {
            "x": np.ascontiguousarray(x[sl]),
            "xbf": np.ascontiguousarray(xbf[:, sl]),
        }
        m.update(folded)
        in_maps.append(m)
    return in_maps


def kernel(**inputs) -> np.ndarray:
    in_maps = make_in_maps(inputs)
    nc = _get_nc()
    res = run_bass_kernel_spmd(nc, in_maps, list(range(N_CORES)))
    out = np.concatenate([res.results[c]["out"] for c in range(N_CORES)], axis=0)
    return out.astype(np.float32)
